# revision 26
# baseline (speedup 1.0000x reference)
"""BNNLinear sampling kernel for Trainium2, data-parallel over 8 NeuronCores.

Computes h[m,c] = sum_r x_ext[m,r] * (mu[c,r] + sqrt(var[c,r]) * E[m,c,r])
with x_ext = concat([x, ones], axis=1), for
  x  [256, 512] f32, mu/var [512, 513] f32, E [256, 512, 513] f32.

Strategy (memory-bound; E is ~269 MB and must stream through HBM once):
 - Shard the sample axis m across the 8 cores (32 samples each).
 - Host-side LAYOUT ONLY: per-sample transpose of E to [r, c] blocked as
   [m, p, k, c] (r = 128k + p) so each per-sample DMA is one contiguous 1 MB
   transfer landing as SBUF tile [128p, 4k, 512c]; mu/var/x are pre-transposed
   the same way (tiny). All arithmetic (sqrt, multiplies, reductions) is
   on-chip.
 - The E stream is split across BOTH HWDGE queues (SP: even samples plus
   the output blocks, Act: odd samples plus the constants): a single HWDGE
   queue tops out ~286 GB/s on this hardware while two queues together
   sustain ~380-430 GB/s. The first samples issue before anything else so
   the stream starts at t~0 and the DMA engines never idle.
 - Per sample: one DVE tensor_tensor B = E_t * sqrt(var)_t ([128, 2048],
   output rounded to f32r), then 5 f32r PE matmuls into a private [1, 512]
   PSUM row: a preload matmul (stationary = identity column, moving = hbs
   [32, 512], start=True) seeds the row with the mean/bias term
   hbs[m,c] = x@mu^T + mu_bias + sqrt(var_bias)*E_bias, then 4 matmuls
   (stationary = x column chunk [128, 1]) accumulate sum_r over the 4
   r-chunks on top.  f32r streams the 512-wide moving operand at 1
   cycle/row (vs 4 for plain f32), keeping the PE far below the DMA
   stream; f32r requires dst partition 0, hence one PSUM row per bank.
 - Each finished PSUM row is drained by an Act copy (DMA cannot read PSUM;
   engine APs must start at partition 0) into [1, 4, C] staging blocks that
   are DMA'd to the DRAM output shard 4 rows at a time.
"""

import numpy as np
from contextlib import ExitStack

import concourse.bacc as bacc
import concourse.mybir as mybir
import concourse.tile as tile
from concourse.bass_utils import run_bass_kernel_spmd

F32 = mybir.dt.float32
F32R = mybir.dt.float32r  # PE fast-fp32 mode: 1 cycle/row (vs 4 for fp32)
                          # when the moving free dim >= 256; same 4-byte data.

N_CORES = 8
M_TOTAL = 256
M_SH = M_TOTAL // N_CORES  # 32 samples per core
C = 512
R_IN = 512                 # r chunks: 4 x 128
KCH = 4

_COMPILED = None


def _build_program(repeat=1):
    nc = bacc.Bacc("TRN2", target_bir_lowering=False, debug=False)

    et_d = nc.dram_tensor("et", [M_SH, 128, KCH, C], F32, kind="ExternalInput").ap()
    eb_d = nc.dram_tensor("eb", [M_SH, C], F32, kind="ExternalInput").ap()
    xt_d = nc.dram_tensor("xt", [128, KCH, M_SH], F32R, kind="ExternalInput").ap()
    mu_d = nc.dram_tensor("mu_t", [128, KCH, C], F32R, kind="ExternalInput").ap()
    mub_d = nc.dram_tensor("mu_b", [1, C], F32, kind="ExternalInput").ap()
    var_d = nc.dram_tensor("var_t", [128, KCH, C], F32, kind="ExternalInput").ap()
    varb_d = nc.dram_tensor("var_b", [1, C], F32, kind="ExternalInput").ap()
    id32_d = nc.dram_tensor("id32", [M_SH, M_SH], F32R, kind="ExternalInput").ap()
    out_d = nc.dram_tensor("out", [M_SH, C], F32, kind="ExternalOutput").ap()

    with tile.TileContext(nc) as tc, ExitStack() as ctx:
        const = ctx.enter_context(tc.tile_pool(name="const", bufs=1))
        work = ctx.enter_context(tc.tile_pool(name="work", bufs=6))
        bpool = ctx.enter_context(tc.tile_pool(name="bpool", bufs=4))
        spool = ctx.enter_context(tc.tile_pool(name="spool", bufs=3))
        psum = ctx.enter_context(tc.tile_pool(name="psum", bufs=6, space="PSUM"))
        psum1 = ctx.enter_context(tc.tile_pool(name="psum1", bufs=1, space="PSUM"))

        # ---- E stream: issue the first sample loads before anything else so
        # the SP queue starts the bulk stream at t~0; the work pool's bufs
        # keep it rolling ahead of compute for the rest of the loop.
        # Steady-state E loads alternate between the SP and Act HWDGE
        # queues: a single queue tops out ~286 GB/s on HW, two sustain ~382.
        n_pre = 6
        pre_tiles = []
        for m in range(n_pre):
            e_t = work.tile([128, KCH, C], F32, tag="et")
            nc.sync.dma_start(e_t[:], et_d[m])
            pre_tiles.append(e_t)

        # ---- constants, all on the Act HWDGE queue (var first: sqrt path)
        var_sb = const.tile([128, KCH, C], F32)
        nc.scalar.dma_start(var_sb[:], var_d)
        xt_sb = const.tile([128, KCH, M_SH], F32R)
        nc.scalar.dma_start(xt_sb[:], xt_d)
        id32_sb = const.tile([M_SH, M_SH], F32R)
        nc.scalar.dma_start(id32_sb[:], id32_d)
        varb_sb = const.tile([1, C], F32)
        nc.scalar.dma_start(varb_sb[:], varb_d)
        mu_sb = const.tile([128, KCH, C], F32R)
        nc.scalar.dma_start(mu_sb[:], mu_d)
        mub_sb = const.tile([1, C], F32)
        nc.scalar.dma_start(mub_sb[:], mub_d)
        eb_sb = const.tile([M_SH, C], F32)
        nc.scalar.dma_start(eb_sb[:], eb_d)

        s_sb = const.tile([128, KCH, C], F32)
        nc.scalar.sqrt(s_sb[:], var_sb[:])
        sb_sb = const.tile([1, C], F32)
        nc.scalar.sqrt(sb_sb[:], varb_sb[:])

        ones32 = const.tile([1, M_SH], F32)
        nc.vector.memset(ones32[:], 1.0)

        # broadcast sqrt(var) bias row to 32 partitions via PE outer product
        ps_b = psum1.tile([M_SH, C], F32)
        nc.tensor.matmul(ps_b[:], lhsT=ones32[:], rhs=sb_sb[:], start=True, stop=True)
        sbb_sb = const.tile([M_SH, C], F32)
        nc.scalar.copy(sbb_sb[:], ps_b[:])

        # h1 = x_t @ mu_t + mu bias row  -> [32, 512] psum, rows = samples
        h1_ps = psum1.tile([M_SH, C], F32)
        for k in range(KCH):
            nc.tensor.matmul(
                h1_ps[:],
                lhsT=xt_sb[:, k, :],
                rhs=mu_sb[:, k, :],
                start=(k == 0), stop=False,
            )
        nc.tensor.matmul(h1_ps[:], lhsT=ones32[:], rhs=mub_sb[:], start=False, stop=True)

        # hbs[m, c] = h1[m, c] + Eb[m, c] * sqrt(var)[c, 512]   (stored f32r:
        # it re-enters the PE as the moving operand of the preload matmul)
        ebs_sb = const.tile([M_SH, C], F32)
        nc.vector.tensor_tensor(
            out=ebs_sb[:], in0=eb_sb[:], in1=sbb_sb[:], op=mybir.AluOpType.mult
        )
        hbs_sb = const.tile([M_SH, C], F32R)
        nc.vector.tensor_tensor(
            out=hbs_sb[:], in0=h1_ps[:], in1=ebs_sb[:], op=mybir.AluOpType.add
        )

        # ---- main loop over samples ----
        for r_i in range(repeat):
            for m in range(M_SH):
                if r_i == 0 and m < n_pre:
                    e_t = pre_tiles[m]
                else:
                    e_t = work.tile([128, KCH, C], F32, tag="et")
                    (nc.sync if m % 2 == 0 else nc.scalar).dma_start(e_t[:], et_d[m])
                bt = bpool.tile([128, KCH, C], F32R, tag="bt")
                if m == M_SH - 1:
                    # last sample of the round: chunk the multiply so each
                    # matmul can start as soon as its r-chunk is scaled,
                    # shortening the pipeline drain
                    for k in range(KCH):
                        nc.vector.tensor_tensor(
                            out=bt[:, k, :], in0=e_t[:, k, :], in1=s_sb[:, k, :],
                            op=mybir.AluOpType.mult,
                        )
                else:
                    nc.vector.tensor_tensor(
                        out=bt[:], in0=e_t[:], in1=s_sb[:], op=mybir.AluOpType.mult
                    )
                pm = psum.tile([1, C], F32, tag="pm")
                nc.tensor.matmul(
                    pm[:], lhsT=id32_sb[:, m : m + 1], rhs=hbs_sb[:],
                    start=True, stop=False, skip_group_check=True,
                )
                for k in range(KCH):
                    nc.tensor.matmul(
                        pm[:],
                        lhsT=xt_sb[:, k, m : m + 1],
                        rhs=bt[:, k, :],
                        start=False,
                        stop=(k == KCH - 1),
                        skip_group_check=True,
                    )
                # drain: Act copy (engine APs must stay at partition 0;
                # DMA can't read PSUM) into a [1, 4, C] staging block,
                # DMA'd out (Act queue) once 4 rows are in
                if m % 4 == 0:
                    st = spool.tile([1, 4, C], F32, tag="st")
                nc.scalar.copy(st[:, m % 4, :], pm[:])
                if m % 4 == 3:
                    nc.sync.dma_start(out_d[m - 3 : m + 1, :], st[:, :, :])

    nc.compile()
    return nc


def _prep_inputs(x, mu, var, E):
    x = np.ascontiguousarray(x, dtype=np.float32)
    mu = np.ascontiguousarray(mu, dtype=np.float32)
    var = np.ascontiguousarray(var, dtype=np.float32)
    E = np.ascontiguousarray(E, dtype=np.float32)

    # mu/var transposed-blocked: [p, k, c] with r = 128k + p (r < 512)
    def blk(t):
        tt = np.ascontiguousarray(t.T[:R_IN])          # [512, 512] (r, c)
        return np.ascontiguousarray(
            tt.reshape(KCH, 128, C).transpose(1, 0, 2)  # [128, 4, 512]
        )

    mu_t = blk(mu)
    var_t = blk(var)
    mu_b = np.ascontiguousarray(mu[:, R_IN]).reshape(1, C)
    var_b = np.ascontiguousarray(var[:, R_IN]).reshape(1, C)
    id32 = np.eye(M_SH, dtype=np.float32)

    # E per-sample transpose + block: [m, p, k, c], r = 128k + p
    et = np.ascontiguousarray(
        E.transpose(0, 2, 1)[:, :R_IN, :]              # [256, 512(r), 512(c)]
        .reshape(M_TOTAL, KCH, 128, C)
        .transpose(0, 2, 1, 3)                          # [256, 128, 4, 512]
    )
    eb = np.ascontiguousarray(E[:, :, R_IN])            # [256, 512]

    # x transposed-blocked per core: [p, k, m_local]
    in_maps = []
    for core in range(N_CORES):
        sl = slice(core * M_SH, (core + 1) * M_SH)
        xs = x[sl]                                      # [32, 512]
        xt = np.ascontiguousarray(
            xs.T.reshape(KCH, 128, M_SH).transpose(1, 0, 2)  # [128, 4, 32]
        )
        in_maps.append({
            "et": np.ascontiguousarray(et[sl]),
            "eb": np.ascontiguousarray(eb[sl]),
            "xt": xt,
            "mu_t": mu_t,
            "var_t": var_t,
            "mu_b": mu_b,
            "var_b": var_b,
            "id32": id32,
        })
    return in_maps


def kernel(x, mu, var, E, shape=None, _trace=False, **_ignored):
    global _COMPILED
    if _COMPILED is None:
        _COMPILED = _build_program()
    nc = _COMPILED
    in_maps = _prep_inputs(np.asarray(x), np.asarray(mu), np.asarray(var), np.asarray(E))
    res = run_bass_kernel_spmd(
        nc, in_maps, core_ids=list(range(N_CORES)), trace=_trace,
    )
    out = np.concatenate([res.results[i]["out"] for i in range(N_CORES)], axis=0)
    if _trace:
        kernel._last_results = res
    return out


# revision 29
# speedup vs baseline: 1.0368x; 1.0368x over previous
"""BNNLinear sampling kernel for Trainium2, data-parallel over 8 NeuronCores.

Computes h[m,c] = sum_r x_ext[m,r] * (mu[c,r] + sqrt(var[c,r]) * E[m,c,r])
with x_ext = concat([x, ones], axis=1), for
  x  [256, 512] f32, mu/var [512, 513] f32, E [256, 512, 513] f32.

Strategy (memory-bound; E is ~269 MB and must stream through HBM once):
 - Shard the sample axis m across the 8 cores (32 samples each).
 - Host-side LAYOUT ONLY: per-sample transpose of E to [r, c] blocked as
   [m, p, k, c] (r = 128k + p) so each per-sample DMA is one contiguous 1 MB
   transfer landing as SBUF tile [128p, 4k, 512c]; mu/var/x are pre-transposed
   the same way (tiny). All arithmetic (sqrt, multiplies, reductions) is
   on-chip.
 - The E stream is split across BOTH HWDGE queues (SP: even samples plus
   the output blocks, Act: odd samples plus the constants): a single HWDGE
   queue tops out ~286 GB/s on this hardware while two queues together
   sustain ~380-430 GB/s. The first samples issue before anything else so
   the stream starts at t~0 and the DMA engines never idle.
 - Per sample: one DVE tensor_tensor B = E_t * sqrt(var)_t ([128, 2048],
   output rounded to f32r), then 5 f32r PE matmuls into a private [1, 512]
   PSUM row: a preload matmul (stationary = identity column, moving = hbs
   [32, 512], start=True) seeds the row with the mean/bias term
   hbs[m,c] = x@mu^T + mu_bias + sqrt(var_bias)*E_bias, then 4 matmuls
   (stationary = x column chunk [128, 1]) accumulate sum_r over the 4
   r-chunks on top.  f32r streams the 512-wide moving operand at 1
   cycle/row (vs 4 for plain f32), keeping the PE far below the DMA
   stream; f32r requires dst partition 0, hence one PSUM row per bank.
 - Each finished PSUM row is drained by an Act copy (DMA cannot read PSUM;
   engine APs must start at partition 0) into [1, 4, C] staging blocks that
   are DMA'd to the DRAM output shard 4 rows at a time.
"""

import numpy as np
from contextlib import ExitStack

import concourse.bacc as bacc
import concourse.mybir as mybir
import concourse.tile as tile
from concourse.bass_utils import run_bass_kernel_spmd

F32 = mybir.dt.float32
F32R = mybir.dt.float32r  # PE fast-fp32 mode: 1 cycle/row (vs 4 for fp32)
                          # when the moving free dim >= 256; same 4-byte data.

N_CORES = 8
M_TOTAL = 256
M_SH = M_TOTAL // N_CORES  # 32 samples per core
C = 512
R_IN = 512                 # r chunks: 4 x 128
KCH = 4

_COMPILED = None


def _build_program(repeat=1):
    nc = bacc.Bacc("TRN2", target_bir_lowering=False, debug=False)

    et_d = nc.dram_tensor("et", [M_SH, 128, KCH, C], F32, kind="ExternalInput").ap()
    eb_d = nc.dram_tensor("eb", [M_SH, C], F32, kind="ExternalInput").ap()
    xt_d = nc.dram_tensor("xt", [128, KCH, M_SH], F32R, kind="ExternalInput").ap()
    mu_d = nc.dram_tensor("mu_t", [128, KCH, C], F32R, kind="ExternalInput").ap()
    mub_d = nc.dram_tensor("mu_b", [1, C], F32, kind="ExternalInput").ap()
    var_d = nc.dram_tensor("var_t", [128, KCH, C], F32, kind="ExternalInput").ap()
    varb_d = nc.dram_tensor("var_b", [1, C], F32, kind="ExternalInput").ap()
    id32_d = nc.dram_tensor("id32", [M_SH, M_SH], F32R, kind="ExternalInput").ap()
    out_d = nc.dram_tensor("out", [M_SH, C], F32, kind="ExternalOutput").ap()

    with tile.TileContext(nc) as tc, ExitStack() as ctx:
        const = ctx.enter_context(tc.tile_pool(name="const", bufs=1))
        work = ctx.enter_context(tc.tile_pool(name="work", bufs=8))
        bpool = ctx.enter_context(tc.tile_pool(name="bpool", bufs=6))
        spool = ctx.enter_context(tc.tile_pool(name="spool", bufs=4))
        psum = ctx.enter_context(tc.tile_pool(name="psum", bufs=6, space="PSUM"))
        psum1 = ctx.enter_context(tc.tile_pool(name="psum1", bufs=1, space="PSUM"))

        # ---- E stream: issue the first sample loads before anything else so
        # the SP queue starts the bulk stream at t~0; the work pool's bufs
        # keep it rolling ahead of compute for the rest of the loop.
        # Steady-state E loads alternate between the SP and Act HWDGE
        # queues: a single queue tops out ~286 GB/s on HW, two sustain ~382.
        n_pre = 6
        pre_tiles = []
        for m in range(n_pre):
            e_t = work.tile([128, KCH, C], F32, tag="et")
            nc.sync.dma_start(e_t[:], et_d[m])
            pre_tiles.append(e_t)

        # ---- constants, all on the Act HWDGE queue (var first: sqrt path)
        var_sb = const.tile([128, KCH, C], F32)
        nc.scalar.dma_start(var_sb[:], var_d)
        xt_sb = const.tile([128, KCH, M_SH], F32R)
        nc.scalar.dma_start(xt_sb[:], xt_d)
        id32_sb = const.tile([M_SH, M_SH], F32R)
        nc.scalar.dma_start(id32_sb[:], id32_d)
        varb_sb = const.tile([1, C], F32)
        nc.scalar.dma_start(varb_sb[:], varb_d)
        mu_sb = const.tile([128, KCH, C], F32R)
        nc.scalar.dma_start(mu_sb[:], mu_d)
        mub_sb = const.tile([1, C], F32)
        nc.scalar.dma_start(mub_sb[:], mub_d)
        eb_sb = const.tile([M_SH, C], F32)
        nc.scalar.dma_start(eb_sb[:], eb_d)

        s_sb = const.tile([128, KCH, C], F32)
        nc.scalar.sqrt(s_sb[:], var_sb[:])
        sb_sb = const.tile([1, C], F32)
        nc.scalar.sqrt(sb_sb[:], varb_sb[:])

        ones32 = const.tile([1, M_SH], F32)
        nc.vector.memset(ones32[:], 1.0)

        # broadcast sqrt(var) bias row to 32 partitions via PE outer product
        ps_b = psum1.tile([M_SH, C], F32)
        nc.tensor.matmul(ps_b[:], lhsT=ones32[:], rhs=sb_sb[:], start=True, stop=True)
        sbb_sb = const.tile([M_SH, C], F32)
        nc.scalar.copy(sbb_sb[:], ps_b[:])

        # h1 = x_t @ mu_t + mu bias row  -> [32, 512] psum, rows = samples
        h1_ps = psum1.tile([M_SH, C], F32)
        for k in range(KCH):
            nc.tensor.matmul(
                h1_ps[:],
                lhsT=xt_sb[:, k, :],
                rhs=mu_sb[:, k, :],
                start=(k == 0), stop=False,
            )
        nc.tensor.matmul(h1_ps[:], lhsT=ones32[:], rhs=mub_sb[:], start=False, stop=True)

        # hbs[m, c] = h1[m, c] + Eb[m, c] * sqrt(var)[c, 512]   (stored f32r:
        # it re-enters the PE as the moving operand of the preload matmul)
        ebs_sb = const.tile([M_SH, C], F32)
        nc.vector.tensor_tensor(
            out=ebs_sb[:], in0=eb_sb[:], in1=sbb_sb[:], op=mybir.AluOpType.mult
        )
        hbs_sb = const.tile([M_SH, C], F32R)
        nc.vector.tensor_tensor(
            out=hbs_sb[:], in0=h1_ps[:], in1=ebs_sb[:], op=mybir.AluOpType.add
        )

        # ---- main loop over samples ----
        for r_i in range(repeat):
            for m in range(M_SH):
                if r_i == 0 and m < n_pre:
                    e_t = pre_tiles[m]
                else:
                    e_t = work.tile([128, KCH, C], F32, tag="et")
                    (nc.sync if m % 2 == 0 else nc.scalar).dma_start(e_t[:], et_d[m])
                bt = bpool.tile([128, KCH, C], F32R, tag="bt")
                if m == M_SH - 1:
                    # last sample of the round: chunk the multiply so each
                    # matmul can start as soon as its r-chunk is scaled,
                    # shortening the pipeline drain
                    for k in range(KCH):
                        nc.vector.tensor_tensor(
                            out=bt[:, k, :], in0=e_t[:, k, :], in1=s_sb[:, k, :],
                            op=mybir.AluOpType.mult,
                        )
                else:
                    nc.vector.tensor_tensor(
                        out=bt[:], in0=e_t[:], in1=s_sb[:], op=mybir.AluOpType.mult
                    )
                pm = psum.tile([1, C], F32, tag="pm")
                nc.tensor.matmul(
                    pm[:], lhsT=id32_sb[:, m : m + 1], rhs=hbs_sb[:],
                    start=True, stop=False, skip_group_check=True,
                )
                for k in range(KCH):
                    nc.tensor.matmul(
                        pm[:],
                        lhsT=xt_sb[:, k, m : m + 1],
                        rhs=bt[:, k, :],
                        start=False,
                        stop=(k == KCH - 1),
                        skip_group_check=True,
                    )
                # drain: Act copy (engine APs must stay at partition 0;
                # DMA can't read PSUM) into a [1, 4, C] staging block,
                # DMA'd out (Act queue) once 4 rows are in
                if m % 4 == 0:
                    st = spool.tile([1, 4, C], F32, tag="st")
                nc.scalar.copy(st[:, m % 4, :], pm[:])
                if m % 4 == 3:
                    nc.sync.dma_start(out_d[m - 3 : m + 1, :], st[:, :, :])

    nc.compile()
    return nc


def _prep_inputs(x, mu, var, E):
    x = np.ascontiguousarray(x, dtype=np.float32)
    mu = np.ascontiguousarray(mu, dtype=np.float32)
    var = np.ascontiguousarray(var, dtype=np.float32)
    E = np.ascontiguousarray(E, dtype=np.float32)

    # mu/var transposed-blocked: [p, k, c] with r = 128k + p (r < 512)
    def blk(t):
        tt = np.ascontiguousarray(t.T[:R_IN])          # [512, 512] (r, c)
        return np.ascontiguousarray(
            tt.reshape(KCH, 128, C).transpose(1, 0, 2)  # [128, 4, 512]
        )

    mu_t = blk(mu)
    var_t = blk(var)
    mu_b = np.ascontiguousarray(mu[:, R_IN]).reshape(1, C)
    var_b = np.ascontiguousarray(var[:, R_IN]).reshape(1, C)
    id32 = np.eye(M_SH, dtype=np.float32)

    # E per-sample transpose + block: [m, p, k, c], r = 128k + p
    et = np.ascontiguousarray(
        E.transpose(0, 2, 1)[:, :R_IN, :]              # [256, 512(r), 512(c)]
        .reshape(M_TOTAL, KCH, 128, C)
        .transpose(0, 2, 1, 3)                          # [256, 128, 4, 512]
    )
    eb = np.ascontiguousarray(E[:, :, R_IN])            # [256, 512]

    # x transposed-blocked per core: [p, k, m_local]
    in_maps = []
    for core in range(N_CORES):
        sl = slice(core * M_SH, (core + 1) * M_SH)
        xs = x[sl]                                      # [32, 512]
        xt = np.ascontiguousarray(
            xs.T.reshape(KCH, 128, M_SH).transpose(1, 0, 2)  # [128, 4, 32]
        )
        in_maps.append({
            "et": np.ascontiguousarray(et[sl]),
            "eb": np.ascontiguousarray(eb[sl]),
            "xt": xt,
            "mu_t": mu_t,
            "var_t": var_t,
            "mu_b": mu_b,
            "var_b": var_b,
            "id32": id32,
        })
    return in_maps


def kernel(x, mu, var, E, shape=None, _trace=False, **_ignored):
    global _COMPILED
    if _COMPILED is None:
        _COMPILED = _build_program()
    nc = _COMPILED
    in_maps = _prep_inputs(np.asarray(x), np.asarray(mu), np.asarray(var), np.asarray(E))
    res = run_bass_kernel_spmd(
        nc, in_maps, core_ids=list(range(N_CORES)), trace=_trace,
    )
    out = np.concatenate([res.results[i]["out"] for i in range(N_CORES)], axis=0)
    if _trace:
        kernel._last_results = res
    return out
